# revision 67
# baseline (speedup 1.0000x reference)
"""Trainium2 Bass kernel for attribute visual attention.

Computes, for each batch b:
    q      = v @ W_alpha                  # [i, f]
    scores = q @ vf[b]                    # [i, r]
    atten  = softmax(scores, axis=r)
    out[b] = atten @ vf[b].T              # [i, f]

Sharding: data-parallel over batch b across 8 NeuronCores (8 batches per
core); v / W_alpha replicated. All matmuls run in fp16 (full PE rate on
TRN2) with fp32 PSUM accumulation; softmax statistics in fp32.

Layout notes:
- The attend matmul contracts over r, which must live on SBUF partitions
  for both operands; the host passes visual_features twice — [f, r] for
  the scores matmul and pre-transposed [r, f] for the attend matmul. The
  small atten matrix is transposed on-chip on the PE (identity matmul).
- Batches are processed in PAIRS for the scores matmul (rhs = two
  batches side by side, N=392): halves the number of PE instructions and
  stationary-weight loads.
- Bulk input traffic uses SWDGE (gpsimd) in natural program order so the
  DMA engines deliver wave k's tiles before wave k+1's; weights and
  outputs ride HWDGE via the otherwise-idle SP queue. Outputs stream per
  4-f-tile chunk as soon as the attend PSUM drains, keeping the tail
  short.
- Softmax normalization is applied while atten is i-partitioned; the Exp
  activation table is preloaded during the PE warmup so the first wave's
  softmax doesn't stall on LoadActFuncSet.
"""

import numpy as np
from contextlib import ExitStack

import concourse.bass as bass
import concourse.tile as tile
import concourse.bass_utils as bass_utils
from concourse import bacc, mybir

# Problem shapes (hardcoded per contest contract).
B, F, R, I, V = 64, 2048, 196, 312, 300
NCORES = 8
BL = B // NCORES          # 8 batches per core
NPAIR = BL // 2           # 4 batch-pairs per core
FT = F // 128             # 16 f-tiles
I_TILES = ((0, 128), (128, 128), (256, 56))
KV_TILES = ((0, 128), (128, 128), (256, 44))    # v=300
KR_TILES = ((0, 128), (128, 68))                # r=196
RH = R // 2                                     # 98: DoubleRow K pairs
WARMUP = 40

F16 = mybir.dt.float16
F32 = mybir.dt.float32
F8 = mybir.dt.float8e4
DR = mybir.MatmulPerfMode.DoubleRow

_CACHE = {}


def _build_body(nc, tc, ctx, wa, vt, vf, vfthi, vftlo, ident, out, reps):
    qtp = ctx.enter_context(tc.tile_pool(name="qt", bufs=1))
    ident_t = qtp.tile([128, 128], F16, tag="ident", name="ident")
    wzero = qtp.tile([128, 128], F16, tag="wzero", name="wzero")
    dummy = qtp.tile([1, 1], F32, tag="dummy", name="dummy")
    # Warm-up operand built on-chip so the PE can start ~0.3us in, long
    # before any DMA lands.
    nc.gpsimd.memset(wzero[:], 0.0)

    # PE warm-up: junk matmuls while the weight loads are still in
    # flight, so the clock ramp completes before real work starts
    with tc.tile_pool(name="wupsum", bufs=1, space=bass.MemorySpace.PSUM) as wup:
        wu = wup.tile([128, 128], F32, tag="wu", name="wu")
        for w in range(WARMUP):
            nc.tensor.matmul(wu[:], wzero[:], wzero[:],
                             start=(w == 0), stop=(w == WARMUP - 1))

    # ---- Phase 0: qT[f, i] = (v @ W_alpha).T via lhsT=W_alpha, rhs=v.T ----
    # const stays open for the whole kernel: if it closed, the vf/vft pools
    # would reuse its SBUF range and wave-0's input DMA would inherit an
    # anti-dependency on qproj's last weight read.
    # vt/ident ride the scalar HWDGE queue, wa the SP queue: the two gen
    # pipelines interleave so all weights land by ~5.5us.
    qt_t = []
    const = ctx.enter_context(tc.tile_pool(name="const", bufs=1))
    with tc.tile_pool(name="qpsum", bufs=6, space=bass.MemorySpace.PSUM) as qpsum:
        wa_t, vt_t = [], []
        with tc.high_priority():
            for k, (v0, vs) in enumerate(KV_TILES):
                t = const.tile([vs, I], F16, tag=f"vt{k}")
                nc.scalar.dma_start(t[:], vt[v0:v0 + vs, :])
                vt_t.append(t)
            for k, (v0, vs) in enumerate(KV_TILES):
                w = const.tile([vs, F], F16, tag=f"wa{k}")
                nc.sync.dma_start(w[:], wa[v0:v0 + vs, :])
                wa_t.append(w)
            nc.sync.dma_start(ident_t[:], ident[:])

        # Preload the Exp activation table while the Act engine is idle
        nc.scalar.activation(dummy[:], wzero[0:1, 0:1],
                             mybir.ActivationFunctionType.Exp)
        # fp8 copy of the identity for the fp8 atten transposes
        ident8 = qtp.tile([128, 128], F8, tag="ident8", name="ident8")
        nc.scalar.copy(ident8[:], ident_t[:])

        # k-outer in groups of 6/5/5 f-tiles: the first group's k=0 stage
        # needs only wa0+vt0, so qproj starts as soon as the first weight
        # tile lands rather than after all three. qt copies alternate
        # Act/DVE so the drain doesn't serialize on one engine.
        groups = ((0, 6), (6, 6), (12, 4))
        for g0, gn in groups:
            qps = [qpsum.tile([128, I], F32, tag="qp", name=f"qp{m}")
                   for m in range(gn)]
            for k in range(3):
                for m in range(gn):
                    mf = g0 + m
                    nc.tensor.matmul(qps[m][:],
                                     wa_t[k][:, mf * 128:(mf + 1) * 128],
                                     vt_t[k][:], start=(k == 0), stop=(k == 2))
            for m in range(gn):
                q = qtp.tile([128, I], F16, tag=f"qt{g0 + m}")
                if m % 2 == 0:
                    nc.scalar.copy(q[:], qps[m][:])
                else:
                    nc.vector.tensor_copy(q[:], qps[m][:])
                qt_t.append(q)

    # ---- Phase 1: per batch-pair attention ----
    vfp = ctx.enter_context(tc.tile_pool(name="vf", bufs=4))
    vftp = ctx.enter_context(tc.tile_pool(name="vft", bufs=3))
    esp = ctx.enter_context(tc.tile_pool(name="es", bufs=6))
    attp = ctx.enter_context(tc.tile_pool(name="atT", bufs=3))
    outp = ctx.enter_context(tc.tile_pool(name="out", bufs=2))
    stat = ctx.enter_context(tc.tile_pool(name="stat", bufs=8))
    scrp = ctx.enter_context(tc.tile_pool(name="scr", bufs=1))
    spsum = ctx.enter_context(
        tc.tile_pool(name="spsum", bufs=2, space=bass.MemorySpace.PSUM))
    opsum = ctx.enter_context(
        tc.tile_pool(name="opsum", bufs=4, space=bass.MemorySpace.PSUM))
    tpsum = ctx.enter_context(
        tc.tile_pool(name="tpsum", bufs=2, space=bass.MemorySpace.PSUM))

    # Wave 0's inputs ride the in-order SP queue BEHIND the weights, so
    # the front of the DMA engines serves qproj's weights first, then
    # vf0/vft0 — exactly the order the PE consumes them.
    def load_wave0():
        vf_t = vfp.tile([128, FT, 2 * R], F16, tag="vf", name="vf")
        for c in range(4):
            nc.sync.dma_start(vf_t[:, 4 * c:4 * (c + 1), :],
                              vf[0, :, 4 * c:4 * (c + 1), :])
        vft_t = {}
        for j in range(2):
            for hl, src_ in (("h", vfthi), ("l", vftlo)):
                vv = vftp.tile([RH, 2, F], F8, tag=f"vft{hl}{j}",
                               name=f"vft{hl}{j}")
                nc.sync.dma_start(vv[:], src_[j])
                vft_t[(j, hl)] = vv
        return vf_t, vft_t

    wave0 = load_wave0()

    # Hold the Pool engine busy ~9.5us with cheap memsets so the SWDGE
    # streams for waves >= 1 don't steal front-end DMA bandwidth.
    scratch = scrp.tile([128, 128], F32, tag="scratch")
    for _ in range(100):
        nc.gpsimd.memset(scratch[:], 0.0)

    for rep in range(reps):
        for half in range(NPAIR):
            if rep == 0 and half == 0:
                vf_t, vft_t = wave0
            else:
                # Input DMAs in natural priority order: wave k's tiles are
                # generated (and hit the DMA engines) before wave k+1's.
                # Pool bufs provide multi-wave prefetch depth.
                vf_t = vfp.tile([128, FT, 2 * R], F16, tag="vf", name="vf")
                for c in range(4):
                    nc.gpsimd.dma_start(vf_t[:, 4 * c:4 * (c + 1), :],
                                        vf[half, :, 4 * c:4 * (c + 1), :])
                vft_t = {}
                for j in range(2):
                    b = 2 * half + j
                    for hl, src_ in (("h", vfthi), ("l", vftlo)):
                        vv = vftp.tile([RH, 2, F], F8, tag=f"vft{hl}{j}",
                                       name=f"vft{hl}{j}")
                        nc.gpsimd.dma_start(vv[:], src_[b])
                        vft_t[(j, hl)] = vv

            if half > 0:
                # PSUM-free PE activity across the wave boundary keeps the
                # clock-ramp monitor fed on hardware
                for _ in range(10):
                    nc.tensor.ldweights(ident_t[:])

            esT_hi = [attp.tile([RH, 2, I], F8, tag=f"esThi{j}",
                                name=f"esThi{j}") for j in range(2)]
            esT_lo = [attp.tile([RH, 2, I], F8, tag=f"esTlo{j}",
                                name=f"esTlo{j}") for j in range(2)]
            # i-tile order (2, 0, 1): the 56-row remainder's softmax chain
            # drains during the big tiles' scores, so attend j0 waits only
            # on the last big tile's transpose, not a trailing chain. The
            # final wave instead ends on the short i2 chain (order 0,1,2)
            # and the attend below starts early on the ready columns.
            last_wave = (rep == reps - 1 and half == NPAIR - 1)
            for mi in ((0, 1, 2) if last_wave else (2, 0, 1)):
                i0, isz = I_TILES[mi]
                sp = spsum.tile([isz, 2, R], F32, tag="sp", name="sp")
                for kf in range(FT):
                    nc.tensor.matmul(
                        sp[:], qt_t[kf][:, i0:i0 + isz],
                        vf_t[:, kf, :].rearrange("p (j r) -> p j r", j=2),
                        start=(kf == 0), stop=(kf == FT - 1))

                negmax = stat.tile([isz, 2], F32, tag="negmax")
                with tc.high_priority():
                    nc.vector.tensor_reduce(negmax[:], sp[:],
                                            axis=mybir.AxisListType.X,
                                            op=mybir.AluOpType.max, negate=True)
                sums = stat.tile([isz, 2], F32, tag="sums")
                rcp = stat.tile([isz, 2], F32, tag="rcp")
                for j in range(2):
                    es = esp.tile([128, R], F16, tag="es")
                    ath = esp.tile([128, R], F8, tag="ath")
                    atl = esp.tile([128, R], F8, tag="atl")
                    with tc.high_priority():
                        nc.scalar.activation(es[:isz, 0:R], sp[:, j, :],
                                             mybir.ActivationFunctionType.Exp,
                                             bias=negmax[:, j:j + 1],
                                             scale=1.0,
                                             accum_out=sums[:, j:j + 1])
                        nc.vector.reciprocal(rcp[:, j:j + 1],
                                             sums[:, j:j + 1])
                        # normalize while atten is still i-partitioned,
                        # quantizing to fp8 hi + residual lo for the
                        # DoubleRow attend (3-term split, ~2e-3 total err)
                        nc.vector.tensor_scalar_mul(ath[:isz, :],
                                                    es[:isz, :],
                                                    rcp[:, j:j + 1])
                        nc.vector.scalar_tensor_tensor(
                            atl[:isz, :], es[:isz, :], rcp[:, j:j + 1],
                            ath[:isz, :], op0=mybir.AluOpType.mult,
                            op1=mybir.AluOpType.subtract)

                    # transpose atten hi/lo -> [rh, p, i-slice] on the PE
                    # (fp8 transpose against the fp8 identity); the psum
                    # drain is a single cheap fp8 copy per parity half
                    for hl, src8, dst8 in (("h", ath, esT_hi),
                                           ("l", atl, esT_lo)):
                        for p in range(2):
                            # hw fp8 transpose writes element-step-2 output
                            tp = tpsum.tile([RH, isz, 2], F8, tag="tp",
                                            name=f"tp{hl}{p}")
                            with tc.high_priority():
                                nc.tensor.transpose(
                                    tp[:, :, 0],
                                    src8[:isz, RH * p:RH * (p + 1)],
                                    ident8[0:isz, 0:isz])
                                if p == 0:
                                    nc.scalar.copy(
                                        dst8[j][:, p, i0:i0 + isz],
                                        tp[:, :, 0])
                                else:
                                    nc.vector.tensor_copy(
                                        dst8[j][:, p, i0:i0 + isz],
                                        tp[:, :, 0])

            # attend (transposed output): outT[f, i] = vfT.T @ attenT,
            # M=f (16 exact tiles), N=i=312 -- no tile waste. Outputs
            # stream to HBM per 4-f-tile chunk as soon as they're copied.
            for j in range(2):
                b = 2 * half + j
                last_batch = (last_wave and j == 1)
                # final chunk of the very last batch is a single f-tile,
                # cutting its DMA transfer latency off the kernel tail
                chunks = (4, 4, 4, 3, 1) if last_batch else (4, 4, 4, 4)
                # last wave's attend j0 starts on esT columns 0:256 (i-tiles
                # 0,1 already transposed) while i2's softmax chain drains
                nsplit = (last_wave and j == 0)
                mf0 = 0
                for c, csz in enumerate(chunks):
                    # per-chunk tiles so each chunk's DMA depends only on
                    # its own copies, not the whole batch
                    otc = outp.tile([128, csz, I], F16,
                                    tag=f"otf{j}c{c}s{csz}",
                                    name=f"otf{j}c{c}s{csz}")
                    for mm in range(csz):
                        mf = mf0 + mm
                        op_ = opsum.tile([128, I], F32, tag="op", name="op")
                        lh = vft_t[(j, "h")][:, :, mf * 128:(mf + 1) * 128]
                        ll = vft_t[(j, "l")][:, :, mf * 128:(mf + 1) * 128]
                        terms = ((lh, esT_hi[j]), (ll, esT_hi[j]),
                                 (lh, esT_lo[j]))
                        if nsplit and mf < 8:
                            for n0, n1 in ((0, 256), (256, I)):
                                for t, (lhs, rhs) in enumerate(terms):
                                    nc.tensor.matmul(
                                        op_[:, n0:n1], lhs, rhs[:, :, n0:n1],
                                        start=(n0 == 0 and t == 0),
                                        stop=(n0 != 0 and t == 2),
                                        perf_mode=DR)
                        else:
                            for t, (lhs, rhs) in enumerate(terms):
                                nc.tensor.matmul(
                                    op_[:], lhs, rhs[:],
                                    start=(t == 0), stop=(t == 2),
                                    perf_mode=DR)
                        if mf % 2 == 0:
                            nc.scalar.copy(otc[:, mm, :], op_[:])
                        else:
                            nc.vector.tensor_copy(otc[:, mm, :], op_[:])
                    nc.sync.dma_start(out[b, :, mf0:mf0 + csz, :], otc[:])
                    mf0 += csz


def _get_program(reps=1):
    key = ("nc", reps)
    if key in _CACHE:
        return _CACHE[key]
    nc = bacc.Bacc("TRN2", target_bir_lowering=False, debug=False,
                   num_devices=NCORES)
    wa_d = nc.dram_tensor("walpha", [V, F], F16, kind="ExternalInput")
    vt_d = nc.dram_tensor("vt", [V, I], F16, kind="ExternalInput")
    vf_d = nc.dram_tensor("vf", [NPAIR, 128, FT, 2 * R], F16,
                          kind="ExternalInput")
    vfthi_d = nc.dram_tensor("vfthi", [BL, RH, 2, F], F8,
                             kind="ExternalInput")
    vftlo_d = nc.dram_tensor("vftlo", [BL, RH, 2, F], F8,
                             kind="ExternalInput")
    id_d = nc.dram_tensor("ident", [128, 128], F16, kind="ExternalInput")
    out_d = nc.dram_tensor("out", [BL, 128, FT, I], F16,
                           kind="ExternalOutput")

    with tile.TileContext(nc) as tc, ExitStack() as ctx:
        _build_body(nc, tc, ctx, wa_d.ap(), vt_d.ap(), vf_d.ap(),
                    vfthi_d.ap(), vftlo_d.ap(), id_d.ap(), out_d.ap(), reps)
    nc.compile()
    _CACHE[key] = nc
    return nc


def _prep_inputs(visual_features, v, W_alpha):
    vf = np.asarray(visual_features, dtype=np.float32)
    v = np.asarray(v, dtype=np.float32)
    W = np.asarray(W_alpha, dtype=np.float32)

    walpha16 = np.ascontiguousarray(W).astype(np.float16)          # [V, F]
    vt16 = np.ascontiguousarray(v.T).astype(np.float16)            # [V, I]
    # [b, f, r] -> [bp, p=128, t=16, j*196+r]: batch-paired, per-partition
    # contiguous DMA layout
    vf16 = np.ascontiguousarray(
        vf.reshape(B // 2, 2, FT, 128, R).transpose(0, 3, 2, 1, 4)
        .reshape(B // 2, 128, FT, 2 * R)).astype(np.float16)
    # vfT packed for DoubleRow: [b, rh, p, f] with r = 98p + rh, as fp8
    # hi + residual lo (3-term split keeps attend error ~2e-3)
    E4 = mybir.dt.np(F8)
    vftp_ = (vf.transpose(0, 2, 1)
             .reshape(B, 2, RH, F).transpose(0, 2, 1, 3))
    vfthi8 = np.ascontiguousarray(vftp_).astype(E4)
    vftlo8 = np.ascontiguousarray(
        vftp_ - vfthi8.astype(np.float32)).astype(E4)

    in_maps = []
    for c in range(NCORES):
        in_maps.append({
            "walpha": walpha16,
            "vt": vt16,
            "ident": np.eye(128, dtype=np.float16),
            "vf": np.ascontiguousarray(vf16[c * NPAIR:(c + 1) * NPAIR]),
            "vfthi": np.ascontiguousarray(vfthi8[c * BL:(c + 1) * BL]),
            "vftlo": np.ascontiguousarray(vftlo8[c * BL:(c + 1) * BL]),
        })
    return in_maps


def kernel(visual_features, v, W_alpha):
    nc = _get_program()
    in_maps = _prep_inputs(visual_features, v, W_alpha)
    res = None
    for attempt in range(3):
        try:
            res = bass_utils.run_bass_kernel_spmd(
                nc, in_maps, core_ids=list(range(NCORES)))
            break
        except Exception:
            # transient NRT_EXEC_UNIT_UNRECOVERABLE wedges have been seen on
            # this fabric; a re-dispatch typically succeeds
            if attempt == 2:
                raise
    outs = [res.results[c]["out"] for c in range(NCORES)]
    buf = np.concatenate(outs, axis=0)          # [B, p=128, t=16, I]
    full = buf.transpose(0, 3, 2, 1).reshape(B, I, F)   # f = t*128 + p
    return np.ascontiguousarray(full).astype(np.float32)


# revision 68
# speedup vs baseline: 1.0028x; 1.0028x over previous
"""Trainium2 Bass kernel for attribute visual attention.

Computes, for each batch b:
    q      = v @ W_alpha                  # [i, f]
    scores = q @ vf[b]                    # [i, r]
    atten  = softmax(scores, axis=r)
    out[b] = atten @ vf[b].T              # [i, f]

Sharding: data-parallel over batch b across 8 NeuronCores (8 batches per
core); v / W_alpha replicated. All matmuls run in fp16 (full PE rate on
TRN2) with fp32 PSUM accumulation; softmax statistics in fp32.

Layout notes:
- The attend matmul contracts over r, which must live on SBUF partitions
  for both operands; the host passes visual_features twice — [f, r] for
  the scores matmul and pre-transposed [r, f] for the attend matmul. The
  small atten matrix is transposed on-chip on the PE (identity matmul).
- Batches are processed in PAIRS for the scores matmul (rhs = two
  batches side by side, N=392): halves the number of PE instructions and
  stationary-weight loads.
- Bulk input traffic uses SWDGE (gpsimd) in natural program order so the
  DMA engines deliver wave k's tiles before wave k+1's; weights and
  outputs ride HWDGE via the otherwise-idle SP queue. Outputs stream per
  4-f-tile chunk as soon as the attend PSUM drains, keeping the tail
  short.
- Softmax normalization is applied while atten is i-partitioned; the Exp
  activation table is preloaded during the PE warmup so the first wave's
  softmax doesn't stall on LoadActFuncSet.
"""

import numpy as np
from contextlib import ExitStack

import concourse.bass as bass
import concourse.tile as tile
import concourse.bass_utils as bass_utils
from concourse import bacc, mybir

# Problem shapes (hardcoded per contest contract).
B, F, R, I, V = 64, 2048, 196, 312, 300
NCORES = 8
BL = B // NCORES          # 8 batches per core
NPAIR = BL // 2           # 4 batch-pairs per core
FT = F // 128             # 16 f-tiles
I_TILES = ((0, 128), (128, 128), (256, 56))
KV_TILES = ((0, 128), (128, 128), (256, 44))    # v=300
KR_TILES = ((0, 128), (128, 68))                # r=196
RH = R // 2                                     # 98: DoubleRow K pairs
WARMUP = 40

F16 = mybir.dt.float16
F32 = mybir.dt.float32
F8 = mybir.dt.float8e4
DR = mybir.MatmulPerfMode.DoubleRow

_CACHE = {}


def _build_body(nc, tc, ctx, wa, vt, vf, vfthi, vftlo, ident, out, reps):
    qtp = ctx.enter_context(tc.tile_pool(name="qt", bufs=1))
    ident_t = qtp.tile([128, 128], F16, tag="ident", name="ident")
    wzero = qtp.tile([128, 128], F16, tag="wzero", name="wzero")
    dummy = qtp.tile([1, 1], F32, tag="dummy", name="dummy")
    # Warm-up operand built on-chip so the PE can start ~0.3us in, long
    # before any DMA lands.
    nc.gpsimd.memset(wzero[:], 0.0)

    # PE warm-up: junk matmuls while the weight loads are still in
    # flight, so the clock ramp completes before real work starts
    with tc.tile_pool(name="wupsum", bufs=1, space=bass.MemorySpace.PSUM) as wup:
        wu = wup.tile([128, 128], F32, tag="wu", name="wu")
        for w in range(WARMUP):
            nc.tensor.matmul(wu[:], wzero[:], wzero[:],
                             start=(w == 0), stop=(w == WARMUP - 1))

    # ---- Phase 0: qT[f, i] = (v @ W_alpha).T via lhsT=W_alpha, rhs=v.T ----
    # const stays open for the whole kernel: if it closed, the vf/vft pools
    # would reuse its SBUF range and wave-0's input DMA would inherit an
    # anti-dependency on qproj's last weight read.
    # vt/ident ride the scalar HWDGE queue, wa the SP queue: the two gen
    # pipelines interleave so all weights land by ~5.5us.
    qt_t = []
    const = ctx.enter_context(tc.tile_pool(name="const", bufs=1))
    with tc.tile_pool(name="qpsum", bufs=6, space=bass.MemorySpace.PSUM) as qpsum:
        wa_t, vt_t = [], []
        with tc.high_priority():
            for k, (v0, vs) in enumerate(KV_TILES):
                t = const.tile([vs, I], F16, tag=f"vt{k}")
                nc.scalar.dma_start(t[:], vt[v0:v0 + vs, :])
                vt_t.append(t)
            for k, (v0, vs) in enumerate(KV_TILES):
                w = const.tile([vs, F], F16, tag=f"wa{k}")
                nc.sync.dma_start(w[:], wa[v0:v0 + vs, :])
                wa_t.append(w)
            nc.sync.dma_start(ident_t[:], ident[:])

        # Preload the Exp activation table while the Act engine is idle
        nc.scalar.activation(dummy[:], wzero[0:1, 0:1],
                             mybir.ActivationFunctionType.Exp)
        # fp8 copy of the identity for the fp8 atten transposes
        ident8 = qtp.tile([128, 128], F8, tag="ident8", name="ident8")
        nc.scalar.copy(ident8[:], ident_t[:])

        # k-outer in groups of 6/5/5 f-tiles: the first group's k=0 stage
        # needs only wa0+vt0, so qproj starts as soon as the first weight
        # tile lands rather than after all three. qt copies alternate
        # Act/DVE so the drain doesn't serialize on one engine.
        groups = ((0, 6), (6, 6), (12, 4))
        for g0, gn in groups:
            qps = [qpsum.tile([128, I], F32, tag="qp", name=f"qp{m}")
                   for m in range(gn)]
            for k in range(3):
                for m in range(gn):
                    mf = g0 + m
                    nc.tensor.matmul(qps[m][:],
                                     wa_t[k][:, mf * 128:(mf + 1) * 128],
                                     vt_t[k][:], start=(k == 0), stop=(k == 2))
            for m in range(gn):
                q = qtp.tile([128, I], F16, tag=f"qt{g0 + m}")
                if m % 2 == 0:
                    nc.scalar.copy(q[:], qps[m][:])
                else:
                    nc.vector.tensor_copy(q[:], qps[m][:])
                qt_t.append(q)

    # ---- Phase 1: per batch-pair attention ----
    vfp = ctx.enter_context(tc.tile_pool(name="vf", bufs=4))
    vftp = ctx.enter_context(tc.tile_pool(name="vft", bufs=3))
    esp = ctx.enter_context(tc.tile_pool(name="es", bufs=6))
    attp = ctx.enter_context(tc.tile_pool(name="atT", bufs=3))
    outp = ctx.enter_context(tc.tile_pool(name="out", bufs=2))
    stat = ctx.enter_context(tc.tile_pool(name="stat", bufs=8))
    scrp = ctx.enter_context(tc.tile_pool(name="scr", bufs=1))
    spsum = ctx.enter_context(
        tc.tile_pool(name="spsum", bufs=2, space=bass.MemorySpace.PSUM))
    opsum = ctx.enter_context(
        tc.tile_pool(name="opsum", bufs=4, space=bass.MemorySpace.PSUM))
    tpsum = ctx.enter_context(
        tc.tile_pool(name="tpsum", bufs=2, space=bass.MemorySpace.PSUM))

    # Wave 0's inputs ride the in-order SP queue BEHIND the weights, so
    # the front of the DMA engines serves qproj's weights first, then
    # vf0/vft0 — exactly the order the PE consumes them.
    def load_wave0():
        vf_t = vfp.tile([128, FT, 2 * R], F16, tag="vf", name="vf")
        for c in range(4):
            nc.sync.dma_start(vf_t[:, 4 * c:4 * (c + 1), :],
                              vf[0, :, 4 * c:4 * (c + 1), :])
        vft_t = {}
        for j in range(2):
            for hl, src_ in (("h", vfthi), ("l", vftlo)):
                vv = vftp.tile([RH, 2, F], F8, tag=f"vft{hl}{j}",
                               name=f"vft{hl}{j}")
                nc.sync.dma_start(vv[:], src_[j])
                vft_t[(j, hl)] = vv
        return vf_t, vft_t

    wave0 = load_wave0()

    # Hold the Pool engine busy ~9.5us with cheap memsets so the SWDGE
    # streams for waves >= 1 don't steal front-end DMA bandwidth.
    scratch = scrp.tile([128, 128], F32, tag="scratch")
    for _ in range(100):
        nc.gpsimd.memset(scratch[:], 0.0)

    for rep in range(reps):
        for half in range(NPAIR):
            if rep == 0 and half == 0:
                vf_t, vft_t = wave0
            else:
                # Input DMAs in natural priority order: wave k's tiles are
                # generated (and hit the DMA engines) before wave k+1's.
                # Pool bufs provide multi-wave prefetch depth.
                vf_t = vfp.tile([128, FT, 2 * R], F16, tag="vf", name="vf")
                for c in range(4):
                    nc.gpsimd.dma_start(vf_t[:, 4 * c:4 * (c + 1), :],
                                        vf[half, :, 4 * c:4 * (c + 1), :])
                vft_t = {}
                for j in range(2):
                    b = 2 * half + j
                    for hl, src_ in (("h", vfthi), ("l", vftlo)):
                        vv = vftp.tile([RH, 2, F], F8, tag=f"vft{hl}{j}",
                                       name=f"vft{hl}{j}")
                        nc.gpsimd.dma_start(vv[:], src_[b])
                        vft_t[(j, hl)] = vv

            if half > 0:
                # PSUM-free PE activity across the wave boundary keeps the
                # clock-ramp monitor fed on hardware
                for _ in range(10):
                    nc.tensor.ldweights(ident_t[:])

            esT_hi = [attp.tile([RH, 2, I], F8, tag=f"esThi{j}",
                                name=f"esThi{j}") for j in range(2)]
            esT_lo = [attp.tile([RH, 2, I], F8, tag=f"esTlo{j}",
                                name=f"esTlo{j}") for j in range(2)]
            # i-tile order (2, 0, 1): the 56-row remainder's softmax chain
            # drains during the big tiles' scores, so attend j0 waits only
            # on the last big tile's transpose, not a trailing chain. The
            # final wave instead ends on the short i2 chain (order 0,1,2)
            # and the attend below starts early on the ready columns.
            last_wave = (rep == reps - 1 and half == NPAIR - 1)
            for mi in ((0, 1, 2) if last_wave else (2, 0, 1)):
                i0, isz = I_TILES[mi]
                sp = spsum.tile([isz, 2, R], F32, tag="sp", name="sp")
                for kf in range(FT):
                    nc.tensor.matmul(
                        sp[:], qt_t[kf][:, i0:i0 + isz],
                        vf_t[:, kf, :].rearrange("p (j r) -> p j r", j=2),
                        start=(kf == 0), stop=(kf == FT - 1))

                negmax = stat.tile([isz, 2], F32, tag="negmax")
                with tc.high_priority():
                    nc.vector.tensor_reduce(negmax[:], sp[:],
                                            axis=mybir.AxisListType.X,
                                            op=mybir.AluOpType.max, negate=True)
                sums = stat.tile([isz, 2], F32, tag="sums")
                rcp = stat.tile([isz, 2], F32, tag="rcp")
                for j in range(2):
                    es = esp.tile([128, R], F16, tag="es")
                    ath = esp.tile([128, R], F8, tag="ath")
                    atl = esp.tile([128, R], F8, tag="atl")
                    with tc.high_priority():
                        nc.scalar.activation(es[:isz, 0:R], sp[:, j, :],
                                             mybir.ActivationFunctionType.Exp,
                                             bias=negmax[:, j:j + 1],
                                             scale=1.0,
                                             accum_out=sums[:, j:j + 1])
                        nc.vector.reciprocal(rcp[:, j:j + 1],
                                             sums[:, j:j + 1])
                        # normalize while atten is still i-partitioned,
                        # quantizing to fp8 hi + residual lo for the
                        # DoubleRow attend (3-term split, ~2e-3 total err)
                        nc.vector.tensor_scalar_mul(ath[:isz, :],
                                                    es[:isz, :],
                                                    rcp[:, j:j + 1])

                    # transpose atten hi -> [rh, p, i-slice] on the PE
                    # BEFORE issuing the lo residual: the attend's hi-term
                    # matmuls carry a coalesced DVE queue-count wait, so
                    # the hi copies must precede atl in queue order
                    def _lo_split():
                        with tc.high_priority():
                            nc.vector.scalar_tensor_tensor(
                                atl[:isz, :], es[:isz, :], rcp[:, j:j + 1],
                                ath[:isz, :], op0=mybir.AluOpType.mult,
                                op1=mybir.AluOpType.subtract)

                    for hl, src8, dst8 in (("h", ath, esT_hi),
                                           ("l", atl, esT_lo)):
                        if hl == "l":
                            _lo_split()
                        for p in range(2):
                            # hw fp8 transpose writes element-step-2 output
                            tp = tpsum.tile([RH, isz, 2], F8, tag="tp",
                                            name=f"tp{hl}{p}")
                            with tc.high_priority():
                                nc.tensor.transpose(
                                    tp[:, :, 0],
                                    src8[:isz, RH * p:RH * (p + 1)],
                                    ident8[0:isz, 0:isz])
                                if p == 0:
                                    nc.scalar.copy(
                                        dst8[j][:, p, i0:i0 + isz],
                                        tp[:, :, 0])
                                else:
                                    nc.vector.tensor_copy(
                                        dst8[j][:, p, i0:i0 + isz],
                                        tp[:, :, 0])

            # attend (transposed output): outT[f, i] = vfT.T @ attenT,
            # M=f (16 exact tiles), N=i=312 -- no tile waste. Outputs
            # stream to HBM per 4-f-tile chunk as soon as they're copied.
            for j in range(2):
                b = 2 * half + j
                last_batch = (last_wave and j == 1)
                # final chunk of the very last batch is a single f-tile,
                # cutting its DMA transfer latency off the kernel tail
                chunks = (4, 4, 4, 3, 1) if last_batch else (4, 4, 4, 4)
                # last wave's attend j0 starts on esT columns 0:256 (i-tiles
                # 0,1 already transposed) while i2's softmax chain drains
                nsplit = (last_wave and j == 0)
                mf0 = 0
                for c, csz in enumerate(chunks):
                    # per-chunk tiles so each chunk's DMA depends only on
                    # its own copies, not the whole batch
                    otc = outp.tile([128, csz, I], F16,
                                    tag=f"otf{j}c{c}s{csz}",
                                    name=f"otf{j}c{c}s{csz}")
                    for mm in range(csz):
                        mf = mf0 + mm
                        op_ = opsum.tile([128, I], F32, tag="op", name="op")
                        lh = vft_t[(j, "h")][:, :, mf * 128:(mf + 1) * 128]
                        ll = vft_t[(j, "l")][:, :, mf * 128:(mf + 1) * 128]
                        terms = ((lh, esT_hi[j]), (ll, esT_hi[j]),
                                 (lh, esT_lo[j]))
                        if nsplit and mf < 8:
                            for n0, n1 in ((0, 256), (256, I)):
                                for t, (lhs, rhs) in enumerate(terms):
                                    nc.tensor.matmul(
                                        op_[:, n0:n1], lhs, rhs[:, :, n0:n1],
                                        start=(n0 == 0 and t == 0),
                                        stop=(n0 != 0 and t == 2),
                                        perf_mode=DR)
                        else:
                            for t, (lhs, rhs) in enumerate(terms):
                                nc.tensor.matmul(
                                    op_[:], lhs, rhs[:],
                                    start=(t == 0), stop=(t == 2),
                                    perf_mode=DR)
                        if mf % 2 == 0:
                            nc.scalar.copy(otc[:, mm, :], op_[:])
                        else:
                            nc.vector.tensor_copy(otc[:, mm, :], op_[:])
                    nc.sync.dma_start(out[b, :, mf0:mf0 + csz, :], otc[:])
                    mf0 += csz


def _get_program(reps=1):
    key = ("nc", reps)
    if key in _CACHE:
        return _CACHE[key]
    nc = bacc.Bacc("TRN2", target_bir_lowering=False, debug=False,
                   num_devices=NCORES)
    wa_d = nc.dram_tensor("walpha", [V, F], F16, kind="ExternalInput")
    vt_d = nc.dram_tensor("vt", [V, I], F16, kind="ExternalInput")
    vf_d = nc.dram_tensor("vf", [NPAIR, 128, FT, 2 * R], F16,
                          kind="ExternalInput")
    vfthi_d = nc.dram_tensor("vfthi", [BL, RH, 2, F], F8,
                             kind="ExternalInput")
    vftlo_d = nc.dram_tensor("vftlo", [BL, RH, 2, F], F8,
                             kind="ExternalInput")
    id_d = nc.dram_tensor("ident", [128, 128], F16, kind="ExternalInput")
    out_d = nc.dram_tensor("out", [BL, 128, FT, I], F16,
                           kind="ExternalOutput")

    with tile.TileContext(nc) as tc, ExitStack() as ctx:
        _build_body(nc, tc, ctx, wa_d.ap(), vt_d.ap(), vf_d.ap(),
                    vfthi_d.ap(), vftlo_d.ap(), id_d.ap(), out_d.ap(), reps)
    nc.compile()
    _CACHE[key] = nc
    return nc


def _prep_inputs(visual_features, v, W_alpha):
    vf = np.asarray(visual_features, dtype=np.float32)
    v = np.asarray(v, dtype=np.float32)
    W = np.asarray(W_alpha, dtype=np.float32)

    walpha16 = np.ascontiguousarray(W).astype(np.float16)          # [V, F]
    vt16 = np.ascontiguousarray(v.T).astype(np.float16)            # [V, I]
    # [b, f, r] -> [bp, p=128, t=16, j*196+r]: batch-paired, per-partition
    # contiguous DMA layout
    vf16 = np.ascontiguousarray(
        vf.reshape(B // 2, 2, FT, 128, R).transpose(0, 3, 2, 1, 4)
        .reshape(B // 2, 128, FT, 2 * R)).astype(np.float16)
    # vfT packed for DoubleRow: [b, rh, p, f] with r = 98p + rh, as fp8
    # hi + residual lo (3-term split keeps attend error ~2e-3)
    E4 = mybir.dt.np(F8)
    vftp_ = (vf.transpose(0, 2, 1)
             .reshape(B, 2, RH, F).transpose(0, 2, 1, 3))
    vfthi8 = np.ascontiguousarray(vftp_).astype(E4)
    vftlo8 = np.ascontiguousarray(
        vftp_ - vfthi8.astype(np.float32)).astype(E4)

    in_maps = []
    for c in range(NCORES):
        in_maps.append({
            "walpha": walpha16,
            "vt": vt16,
            "ident": np.eye(128, dtype=np.float16),
            "vf": np.ascontiguousarray(vf16[c * NPAIR:(c + 1) * NPAIR]),
            "vfthi": np.ascontiguousarray(vfthi8[c * BL:(c + 1) * BL]),
            "vftlo": np.ascontiguousarray(vftlo8[c * BL:(c + 1) * BL]),
        })
    return in_maps


def kernel(visual_features, v, W_alpha):
    nc = _get_program()
    in_maps = _prep_inputs(visual_features, v, W_alpha)
    res = None
    for attempt in range(3):
        try:
            res = bass_utils.run_bass_kernel_spmd(
                nc, in_maps, core_ids=list(range(NCORES)))
            break
        except Exception:
            # transient NRT_EXEC_UNIT_UNRECOVERABLE wedges have been seen on
            # this fabric; a re-dispatch typically succeeds
            if attempt == 2:
                raise
    outs = [res.results[c]["out"] for c in range(NCORES)]
    buf = np.concatenate(outs, axis=0)          # [B, p=128, t=16, I]
    full = buf.transpose(0, 3, 2, 1).reshape(B, I, F)   # f = t*128 + p
    return np.ascontiguousarray(full).astype(np.float32)
